# revision 6
# baseline (speedup 1.0000x reference)
"""Bass/Tile kernel for nn_Attention_49959059587521 on 8 TRN2 NeuronCores.

Math per (batch b, head h), with Q,K,V,Q2,K2 = [2048, 64] slices:
    S    = (Q @ K^T) * 0.125                    # [2048, 2048]
    P    = softmax(S, axis=-1)
    gate = sigmoid((Q2 @ sum_n(K2)) * 0.125)    # [2048]
    out  = (P * gate[:, None]) @ V              # [2048, 64]

Sharding: 32 (b, h) pairs over 8 cores -> core i handles b = i//2 and the 4
heads h in [4*(i%2), 4*(i%2)+4), i.e. the channel slice [256*(i%2), +256).
No cross-core communication.

Per-core algorithm (fully on device):
  - S^T[k, q] = K^T(stationary) x Q^T(moving) via bf16 matmuls. Heads are
    processed in stacked pairs so every matmul contracts over K=128
    partitions (K=64 streams at half rate and K-switches stall the PE);
    each head's K^T weights are zero-padded on the other head's 64
    partitions.
  - exp fused on ScalarE reading PSUM directly (scale=0.125 via free affine);
    no max-subtraction needed: logits are ~N(0,1), |S| < ~7, exp is safe in f32.
  - O^T = V'^T @ E accumulated in PSUM over the 16 k-tiles, where V' = [V; ones]
    so row 64 of O^T is the softmax denominator (free rowsum).
  - gate computed as 1/(1+exp(-z*scale)) (exp table only; inf-safe).
  - O^T 128-column blocks PE-transposed back to [q, d]; the PSUM->SBUF copy
    fuses the (gate * 1/rowsum) per-row scale on the VectorE.

Scheduling notes (the ScalarE exp stream, 128 x ~1.07us, is the hard floor;
everything else is arranged to keep it saturated from ~10us on):
  - identity built on GpSimd BEFORE the DMA triggers (PE transposes were
    otherwise blocked ~10us behind the trigger queue).
  - input DMAs are priority-ordered and spread over three DGE queues:
    Pool (SW-DGE): q01, k0-3, v1, v3, q2 (first-needed first); SP: v0, v2;
    ACT: k2 (single 2MB transfer, needed only at ~30us for the gate).
  - both head-pairs' transposes are emitted per cast-group, so they all run
    in the startup window instead of stalling the exp stream at the j=0/j=1
    boundary.
  - gate factors for all 4 heads are computed once (single exp) as soon as
    k2/q2 land; phase2 (O^T transpose + scale + store) is emitted per-half
    so it drains in the shadow of the next half's exp stream.
"""

import functools
from contextlib import ExitStack

import numpy as np

import concourse.bass as bass
import concourse.mybir as mybir
import concourse.tile as tile
from concourse import bacc, bass_isa, bass_utils
from concourse.masks import make_identity

F32 = mybir.dt.float32

B, NT, C, H = 4, 2048, 512, 8
HD = 64
SCALE = HD ** -0.5  # 0.125
P = 128
NO = NT // P            # 16 n-tiles
NH4 = 4                 # heads per core
CW = NH4 * HD           # 256 channels per core
NHALF = 2               # q processed in two halves of 1024
QH = NT // NHALF        # 1024
BF16 = mybir.dt.bfloat16
MM_DT = BF16            # dtype of matmul operands (qT/kT/V'/E)
U32 = mybir.dt.uint32


def _build(use_sigmoid: bool):
    nc = bacc.Bacc("TRN2", target_bir_lowering=False)
    q_d = nc.dram_tensor("q", [NT, CW], F32, kind="ExternalInput")
    k_d = nc.dram_tensor("k", [NT, CW], F32, kind="ExternalInput")
    v_d = nc.dram_tensor("v", [NT, CW], F32, kind="ExternalInput")
    if use_sigmoid:
        q2_d = nc.dram_tensor("q2", [NT, CW], F32, kind="ExternalInput")
        k2_d = nc.dram_tensor("k2", [NT, CW], F32, kind="ExternalInput")
    out_d = nc.dram_tensor("out", [NT, CW], F32, kind="ExternalOutput")

    with tile.TileContext(nc) as tc, ExitStack() as ctx:
        singles = ctx.enter_context(tc.tile_pool(name="singles", bufs=1))
        tpool = ctx.enter_context(tc.tile_pool(name="tp", bufs=2))
        epool = ctx.enter_context(tc.tile_pool(name="ep", bufs=5))
        opool = ctx.enter_context(tc.tile_pool(name="op", bufs=2))
        # PSUM: st 2x[128,1024] = 4 banks, acc 1x[65,1024] = 2 banks,
        # tr 2x[<=128,<=512] = 2 banks. Total 8 banks.
        ps_st = ctx.enter_context(tc.tile_pool(name="ps_st", bufs=2, space="PSUM"))
        ps_ac = ctx.enter_context(tc.tile_pool(name="ps_ac", bufs=1, space="PSUM"))
        ps_tr = ctx.enter_context(tc.tile_pool(name="ps_tr", bufs=2, space="PSUM"))

        def tr_tile(shape, dtype=F32):
            return ps_tr.tile(shape, dtype, tag="ptr", name="ptr")

        # ---- identities: ident_b FIRST on GpSimd so the PE transposes are
        # unblocked as soon as the first casts land (~4us); the f32 one (for
        # phase2, first needed ~35us) goes after the DMA triggers.
        ident_b = singles.tile([P, P], BF16)
        make_identity(nc, ident_b)

        # ---- input tiles ([n, c] -> [p, o, c] tiling) ----
        q_sb = singles.tile([P, NO, CW], F32, name="q_sb", tag="q_sb")
        k_sb = singles.tile([P, NO, CW], F32, name="k_sb", tag="k_sb")
        v_sb = singles.tile([P, NO, CW], F32, name="v_sb", tag="v_sb")
        q_src = q_d.ap().rearrange("(o p) c -> p o c", p=P)
        k_src = k_d.ap().rearrange("(o p) c -> p o c", p=P)
        v_src = v_d.ap().rearrange("(o p) c -> p o c", p=P)

        def g_sl(g):
            return slice(4 * g, 4 * (g + 1))

        # Pool SW-DGE triggers (~0.9us each to issue, transfers then run
        # concurrently at ~80GB/s per channel): first-needed first.
        # st(0) needs q groups 0,1 + k group 0; k group t//4 gates st(t);
        # v group g gates acc(4g..4g+3); q groups 2,3 only gate half 1
        # (~+18us); q2 only gates the gate path (~+25us).
        nc.gpsimd.dma_start(q_sb[:, g_sl(0), :], q_src[:, g_sl(0), :])
        nc.gpsimd.dma_start(q_sb[:, g_sl(1), :], q_src[:, g_sl(1), :])
        nc.gpsimd.dma_start(k_sb[:, g_sl(0), :], k_src[:, g_sl(0), :])
        nc.gpsimd.dma_start(k_sb[:, g_sl(1), :], k_src[:, g_sl(1), :])
        nc.gpsimd.dma_start(v_sb[:, g_sl(1), :], v_src[:, g_sl(1), :])
        nc.gpsimd.dma_start(k_sb[:, g_sl(2), :], k_src[:, g_sl(2), :])
        nc.gpsimd.dma_start(k_sb[:, g_sl(3), :], k_src[:, g_sl(3), :])
        nc.gpsimd.dma_start(v_sb[:, g_sl(3), :], v_src[:, g_sl(3), :])
        nc.gpsimd.dma_start(q_sb[:, g_sl(2), :], q_src[:, g_sl(2), :])
        nc.gpsimd.dma_start(q_sb[:, g_sl(3), :], q_src[:, g_sl(3), :])
        # gate inputs last: first needed ~25us in, and they must stay behind
        # the hot loads in BOTH the scheduler's model and reality (a single
        # big early transfer on a HW-DGE queue once serialized the whole DVE
        # order behind a 25us k2 wait).
        if use_sigmoid:
            q2_sb = singles.tile([P, NO, CW], F32, name="q2_sb", tag="q2_sb")
            k2_sb = singles.tile([P, NO, CW], F32, name="k2_sb", tag="k2_sb")
            q2_src = q2_d.ap().rearrange("(o p) c -> p o c", p=P)
            k2_src = k2_d.ap().rearrange("(o p) c -> p o c", p=P)
            nc.gpsimd.dma_start(q2_sb[:, 0:8, :], q2_src[:, 0:8, :])
            nc.gpsimd.dma_start(q2_sb[:, 8:16, :], q2_src[:, 8:16, :])
            for g in range(4):
                nc.gpsimd.dma_start(k2_sb[:, g_sl(g), :], k2_src[:, g_sl(g), :])

        # SP HW-DGE: v groups 0 and 2 (serial on the SP channel: ~7.5us and
        # ~14us arrival — in time for acc t=0..3 / t=8..11).
        nc.sync.dma_start(v_sb[:, g_sl(0), :], v_src[:, g_sl(0), :])
        nc.sync.dma_start(v_sb[:, g_sl(2), :], v_src[:, g_sl(2), :])

        # f32 identity + remaining weight-tile zero-halves on GpSimd, after
        # the triggers (first consumer of kTzb0 is head 1 at ~30us; of pair 1
        # head 2 at ~50us; ident at ~35us).
        kTz_all = []
        for jp in range(NH4 // 2):
            kTza = tpool.tile([P, NT], MM_DT, tag="kTza", name="kTza")
            kTzb = tpool.tile([P, NT], MM_DT, tag="kTzb", name="kTzb")
            kTz_all.extend([kTza, kTzb])
        ident = singles.tile([P, P], F32)
        make_identity(nc, ident)
        nc.gpsimd.memset(kTz_all[1][0:HD, :].bitcast(U32), 0)
        nc.gpsimd.memset(kTz_all[2][HD:P, :].bitcast(U32), 0)
        nc.gpsimd.memset(kTz_all[3][0:HD, :].bitcast(U32), 0)

        # V' = [V | ones]: the ones column for all heads in one memset.
        v1r = singles.tile([P, NO, NH4, HD + 1], MM_DT)
        nc.gpsimd.memset(v1r[:, :, :, HD : HD + 1], 1.0)

        # DVE: zero-half of the first weight tile (gates st(0), so it leads
        # the DVE queue), then per-group casts + transpose staging copies.
        nc.vector.memset(kTz_all[0][HD:P, :].bitcast(U32), 0)

        qbf = singles.tile([P, NO, CW], BF16)
        kbf = singles.tile([P, NO, CW], BF16)
        qT2s = [
            tpool.tile([P, NT], MM_DT, tag="qT2", name="qT2") for _ in range(2)
        ]

        # ---- stage B: stacked transposes for BOTH head pairs per cast
        # group: qT2[jp] [128, 2048] holds heads 2jp (partitions 0:64) and
        # 2jp+1 (64:128); kT is split into two zero-padded weight tensors so
        # the logit matmuls contract over the full 128 partitions. Emitting
        # both pairs here lets all of it run in the DMA-bound startup window.
        for g in range(NO // 4):
            gsl = g_sl(g)
            nc.vector.tensor_copy(qbf[:, gsl, :], q_sb[:, gsl, :])
            nc.vector.tensor_copy(kbf[:, gsl, :], k_sb[:, gsl, :])
            for jp in range(2):
                cp = 2 * HD * jp
                tp = tr_tile([P, 4 * P], BF16)
                for u in range(4):
                    o = 4 * g + u
                    nc.tensor.transpose(
                        tp[:, P * u : P * (u + 1)], qbf[:, o, cp : cp + P], ident_b
                    )
                nc.vector.tensor_copy(qT2s[jp][:, 4 * P * g : 4 * P * (g + 1)], tp)
                tp2 = tr_tile([P, 4 * P], BF16)
                for u in range(4):
                    o = 4 * g + u
                    nc.tensor.transpose(
                        tp2[:, P * u : P * (u + 1)], kbf[:, o, cp : cp + P], ident_b
                    )
                nc.vector.tensor_copy(
                    kTz_all[2 * jp][0:HD, 4 * P * g : 4 * P * (g + 1)], tp2[0:HD]
                )
                nc.vector.tensor_copy(
                    kTz_all[2 * jp + 1][HD:P, 4 * P * g : 4 * P * (g + 1)], tp2[HD:P]
                )

        # V' value columns: head 0 per-group (so acc t can start as each v
        # group lands), heads 1-3 as whole copies (DVE backfills ~15-20us).
        for g in range(NO // 4):
            nc.vector.tensor_copy(
                v1r[:, g_sl(g), 0, 0:HD], v_sb[:, g_sl(g), 0:HD]
            )
        for j in range(1, NH4):
            ch = HD * j
            nc.vector.tensor_copy(v1r[:, :, j, 0:HD], v_sb[:, :, ch : ch + HD])

        # ---- gate factors for all heads at once (emitted at head-1
        # priority inside the j-loop, so head 0's startup work always
        # outranks it; data-gates on k2/q2 make it run ~20-30us in, well
        # before the first phase2 reads gte_all) ----
        gte_all = None

        def emit_gate_all():
            k2o = singles.tile([P, CW], F32)
            k2b_sb = singles.tile([P, CW], F32)
            for cc in range(4):
                csl = slice(HD * cc, HD * (cc + 1))
                nc.vector.reduce_sum(
                    out=k2o[:, csl],
                    in_=k2_sb[:, :, csl].rearrange("p o c -> p c o"),
                    axis=mybir.AxisListType.X,
                )
            nc.gpsimd.partition_all_reduce(
                k2b_sb, k2o, channels=P, reduce_op=bass_isa.ReduceOp.add
            )
            z_all = singles.tile([P, NO, NH4], F32)
            for j in range(NH4):
                ch = HD * j
                zt = opool.tile([P, NO, HD], F32, tag="zt", name="zt")
                nc.vector.tensor_mul(
                    zt,
                    q2_sb[:, :, ch : ch + HD],
                    k2b_sb[:, None, ch : ch + HD].to_broadcast((P, NO, HD)),
                )
                nc.vector.reduce_sum(
                    out=z_all[:, :, j], in_=zt, axis=mybir.AxisListType.X
                )
            eg_all = singles.tile([P, NO, NH4], F32)
            nc.scalar.activation(
                eg_all, z_all, mybir.ActivationFunctionType.Exp, scale=-SCALE
            )
            nc.vector.tensor_scalar_add(eg_all, eg_all, 1.0)
            g_t = singles.tile([P, NO, NH4], F32)
            nc.vector.reciprocal(g_t, eg_all)
            return g_t

        out_ap3 = out_d.ap().rearrange("(o p) c -> p o c", p=P)

        # ---- main loop: per (head, half): 16x [st matmuls -> exp -> acc
        # matmuls], then O^T copy-out + phase2 (transpose + gate/rowsum
        # scale + store). phase2/out drain in the shadow of the next half's
        # exp stream; the last half drains at fine grain to shrink the tail.
        for j in range(NH4):  # local head
            jp, jj = divmod(j, 2)
            ch = HD * j
            qT2 = qT2s[jp]
            kTz = kTz_all[2 * jp + jj]
            for h in range(NHALF):  # q half
                last = j == NH4 - 1 and h == NHALF - 1
                q0 = QH * h
                acc = ps_ac.tile([HD + 1, QH], F32, tag="pac")
                for t in range(NO):
                    st = ps_st.tile([P, QH], F32, tag="pst")
                    for s2 in range(QH // 512):
                        nc.tensor.matmul(
                            st[:, 512 * s2 : 512 * (s2 + 1)],
                            kTz[:, P * t : P * (t + 1)],
                            qT2[:, q0 + 512 * s2 : q0 + 512 * (s2 + 1)],
                            start=True,
                            stop=True,
                        )
                    et = epool.tile([P, QH], MM_DT, tag="et")
                    nc.scalar.activation(
                        et, st, mybir.ActivationFunctionType.Exp, scale=SCALE
                    )
                    for s2 in range(QH // 512):
                        nc.tensor.matmul(
                            acc[:, 512 * s2 : 512 * (s2 + 1)],
                            v1r[:, t, j, :],
                            et[:, 512 * s2 : 512 * (s2 + 1)],
                            start=(t == 0),
                            stop=(t == NO - 1),
                        )

                if use_sigmoid and j == 0 and h == 0:
                    # gate chain lands here in priority: below all startup
                    # and (0,h0) work, above everything later; its k2/q2
                    # data-gates let it fill DVE idle during (0,h0)'s
                    # exp-bound stream.
                    gte_all = emit_gate_all()

                # O^T -> SBUF (+ fused rowsum/gate scale after transpose)
                ot_sb = opool.tile([HD + 1, QH], F32, tag="ot", bufs=3, name="ot_sb")
                obuf = opool.tile([P, QH // P, HD], F32, tag="obuf")
                if last:
                    # per-u copies so the drain pipeline starts 1 tile in
                    for u in range(QH // P):
                        nc.vector.tensor_copy(
                            ot_sb[:, P * u : P * (u + 1)], acc[:, P * u : P * (u + 1)]
                        )
                else:
                    nc.vector.tensor_copy(ot_sb, acc)

                chunk = 2 if last else 4  # o-tiles per output DMA
                for u in range(QH // P):
                    i = (QH // P) * h + u
                    tr = tr_tile([P, HD + 1])
                    nc.tensor.transpose(
                        tr, ot_sb[:, P * u : P * (u + 1)], ident[: HD + 1, : HD + 1]
                    )
                    rcp = opool.tile([P, 1], F32, tag="rcp", name="rcp")
                    nc.vector.reciprocal(rcp, tr[:, HD : HD + 1])
                    if use_sigmoid:
                        fac = opool.tile([P, 1], F32, tag="fac", name="fac")
                        nc.vector.tensor_mul(fac, rcp, gte_all[:, i, j : j + 1])
                    else:
                        fac = rcp
                    nc.vector.tensor_scalar_mul(obuf[:, u, :], tr[:, 0:HD], fac)
                    if u % chunk == chunk - 1:
                        c0 = u - chunk + 1
                        nc.sync.dma_start(
                            out_ap3[:, 8 * h + c0 : 8 * h + u + 1, ch : ch + HD],
                            obuf[:, c0 : u + 1, :],
                        )

    nc.compile()
    return nc


@functools.lru_cache(maxsize=2)
def _graph(use_sigmoid: bool):
    return _build(use_sigmoid)


def _shard(a: np.ndarray, i: int) -> np.ndarray:
    b, hg = divmod(i, 2)
    return np.ascontiguousarray(a[b, :, hg * CW : (hg + 1) * CW], dtype=np.float32)


def run(inputs, trace: bool = False):
    use_sigmoid = bool(np.asarray(inputs["use_sigmoid"]).item())
    nc = _graph(use_sigmoid)
    in_maps = []
    for i in range(8):
        m = {
            "q": _shard(np.asarray(inputs["query"]), i),
            "k": _shard(np.asarray(inputs["key"]), i),
            "v": _shard(np.asarray(inputs["value"]), i),
        }
        if use_sigmoid:
            m["q2"] = _shard(np.asarray(inputs["query2"]), i)
            m["k2"] = _shard(np.asarray(inputs["key2"]), i)
        in_maps.append(m)
    res = bass_utils.run_bass_kernel_spmd(
        nc, in_maps, core_ids=list(range(8)), trace=trace
    )
    out = np.empty((B, NT, C), dtype=np.float32)
    for i in range(8):
        b, hg = divmod(i, 2)
        out[b, :, hg * CW : (hg + 1) * CW] = res.results[i]["out"]
    return out, res


def kernel(**inputs) -> np.ndarray:
    out, _ = run(inputs)
    return out


if __name__ == "__main__":
    rng = np.random.default_rng(0)
    fake = {
        "query": rng.standard_normal((B, NT, C), dtype=np.float32),
        "key": rng.standard_normal((B, NT, C), dtype=np.float32),
        "value": rng.standard_normal((B, NT, C), dtype=np.float32),
        "query2": rng.standard_normal((B, NT, C), dtype=np.float32),
        "key2": rng.standard_normal((B, NT, C), dtype=np.float32),
        "use_sigmoid": 1,
    }
    out = kernel(**fake)
    print("ran ok", out.shape, out.dtype)


# revision 7
# speedup vs baseline: 1.1685x; 1.1685x over previous
"""Bass/Tile kernel for nn_Attention_49959059587521 on 8 TRN2 NeuronCores.

Math per (batch b, head h), with Q,K,V,Q2,K2 = [2048, 64] slices:
    S    = (Q @ K^T) * 0.125                    # [2048, 2048]
    P    = softmax(S, axis=-1)
    gate = sigmoid((Q2 @ sum_n(K2)) * 0.125)    # [2048]
    out  = (P * gate[:, None]) @ V              # [2048, 64]

Sharding: 32 (b, h) pairs over 8 cores -> core i handles b = i//2 and the 4
heads h in [4*(i%2), 4*(i%2)+4), i.e. the channel slice [256*(i%2), +256).
No cross-core communication.

Per-core algorithm (fully on device):
  - S^T[k, q] = K^T(stationary) x Q^T(moving) via bf16 matmuls. Heads are
    processed in stacked pairs so every matmul contracts over K=128
    partitions (K=64 streams at half rate and K-switches stall the PE);
    each head's K^T weights are zero-padded on the other head's 64
    partitions.
  - exp fused on ScalarE reading PSUM directly (scale=0.125 via free affine);
    no max-subtraction needed: logits are ~N(0,1), |S| < ~7, exp is safe in f32.
  - O^T = V'^T @ E accumulated in PSUM over the 16 k-tiles, where V' = [V; ones]
    so row 64 of O^T is the softmax denominator (free rowsum).
  - gate computed as 1/(1+exp(-z*scale)) (exp table only; inf-safe).
  - O^T 128-column blocks PE-transposed back to [q, d]; the PSUM->SBUF copy
    fuses the (gate * 1/rowsum) per-row scale on the VectorE.

Scheduling notes (the ScalarE exp stream, 128 x ~1.07us, is the hard floor;
everything else is arranged to keep it saturated from ~10us on):
  - identity built on GpSimd BEFORE the DMA triggers (PE transposes were
    otherwise blocked ~10us behind the trigger queue).
  - input DMAs are priority-ordered and spread over three DGE queues:
    Pool (SW-DGE): q01, k0-3, v1, v3, q2 (first-needed first); SP: v0, v2;
    ACT: k2 (single 2MB transfer, needed only at ~30us for the gate).
  - both head-pairs' transposes are emitted per cast-group, so they all run
    in the startup window instead of stalling the exp stream at the j=0/j=1
    boundary.
  - gate factors for all 4 heads are computed once (single exp) as soon as
    k2/q2 land; phase2 (O^T transpose + scale + store) is emitted per-half
    so it drains in the shadow of the next half's exp stream.
"""

import functools
from contextlib import ExitStack

import numpy as np

import concourse.bass as bass
import concourse.mybir as mybir
import concourse.tile as tile
from concourse import bacc, bass_isa, bass_utils
from concourse.masks import make_identity

F32 = mybir.dt.float32

B, NT, C, H = 4, 2048, 512, 8
HD = 64
SCALE = HD ** -0.5  # 0.125
P = 128
NO = NT // P            # 16 n-tiles
NH4 = 4                 # heads per core
CW = NH4 * HD           # 256 channels per core
NHALF = 2               # q processed in two halves of 1024
QH = NT // NHALF        # 1024
BF16 = mybir.dt.bfloat16
MM_DT = BF16            # dtype of matmul operands (qT/kT/V'/E)
U32 = mybir.dt.uint32


def _build(use_sigmoid: bool):
    nc = bacc.Bacc("TRN2", target_bir_lowering=False)
    q_d = nc.dram_tensor("q", [NT, CW], F32, kind="ExternalInput")
    k_d = nc.dram_tensor("k", [NT, CW], F32, kind="ExternalInput")
    v_d = nc.dram_tensor("v", [NT, CW], F32, kind="ExternalInput")
    if use_sigmoid:
        q2_d = nc.dram_tensor("q2", [NT, CW], F32, kind="ExternalInput")
        k2_d = nc.dram_tensor("k2", [NT, CW], F32, kind="ExternalInput")
    out_d = nc.dram_tensor("out", [NT, CW], F32, kind="ExternalOutput")

    with tile.TileContext(nc) as tc, ExitStack() as ctx:
        singles = ctx.enter_context(tc.tile_pool(name="singles", bufs=1))
        tpool = ctx.enter_context(tc.tile_pool(name="tp", bufs=2))
        epool = ctx.enter_context(tc.tile_pool(name="ep", bufs=4))
        opool = ctx.enter_context(tc.tile_pool(name="op", bufs=2))
        # PSUM: st 2x[128,1024] = 4 banks, acc 1x[65,1024] = 2 banks,
        # tr 2x[<=128,<=512] = 2 banks. Total 8 banks.
        ps_st = ctx.enter_context(tc.tile_pool(name="ps_st", bufs=2, space="PSUM"))
        ps_ac = ctx.enter_context(tc.tile_pool(name="ps_ac", bufs=1, space="PSUM"))
        ps_tr = ctx.enter_context(tc.tile_pool(name="ps_tr", bufs=2, space="PSUM"))

        def tr_tile(shape, dtype=F32):
            return ps_tr.tile(shape, dtype, tag="ptr", name="ptr")

        # ---- identities: ident_b FIRST on GpSimd so the PE transposes are
        # unblocked as soon as the first casts land (~4us); the f32 one (for
        # phase2, first needed ~35us) goes after the DMA triggers.
        ident_b = singles.tile([P, P], BF16)
        make_identity(nc, ident_b)

        # ---- input tiles ([n, c] -> [p, o, c] tiling) ----
        q_sb = singles.tile([P, NO, CW], F32, name="q_sb", tag="q_sb")
        k_sb = singles.tile([P, NO, CW], F32, name="k_sb", tag="k_sb")
        v_sb = singles.tile([P, NO, CW], F32, name="v_sb", tag="v_sb")
        q_src = q_d.ap().rearrange("(o p) c -> p o c", p=P)
        k_src = k_d.ap().rearrange("(o p) c -> p o c", p=P)
        v_src = v_d.ap().rearrange("(o p) c -> p o c", p=P)

        def g_sl(g):
            return slice(4 * g, 4 * (g + 1))

        # Pool SW-DGE triggers (~0.9us each to issue, transfers then run
        # concurrently at ~80GB/s per channel): first-needed first.
        # st(0) needs q groups 0,1 + k group 0; k group t//4 gates st(t);
        # v group g gates acc(4g..4g+3); q groups 2,3 only gate half 1
        # (~+18us); q2 only gates the gate path (~+25us).
        nc.gpsimd.dma_start(q_sb[:, g_sl(0), :], q_src[:, g_sl(0), :])
        nc.gpsimd.dma_start(q_sb[:, g_sl(1), :], q_src[:, g_sl(1), :])
        nc.gpsimd.dma_start(k_sb[:, g_sl(0), :], k_src[:, g_sl(0), :])
        nc.gpsimd.dma_start(k_sb[:, g_sl(1), :], k_src[:, g_sl(1), :])
        nc.gpsimd.dma_start(v_sb[:, g_sl(1), :], v_src[:, g_sl(1), :])
        nc.gpsimd.dma_start(k_sb[:, g_sl(2), :], k_src[:, g_sl(2), :])
        nc.gpsimd.dma_start(k_sb[:, g_sl(3), :], k_src[:, g_sl(3), :])
        nc.gpsimd.dma_start(v_sb[:, g_sl(3), :], v_src[:, g_sl(3), :])
        nc.gpsimd.dma_start(q_sb[:, g_sl(2), :], q_src[:, g_sl(2), :])
        nc.gpsimd.dma_start(q_sb[:, g_sl(3), :], q_src[:, g_sl(3), :])
        # gate inputs last: first needed ~25us in, and they must stay behind
        # the hot loads in BOTH the scheduler's model and reality (a single
        # big early transfer on a HW-DGE queue once serialized the whole DVE
        # order behind a 25us k2 wait).
        if use_sigmoid:
            q2_sb = singles.tile([P, NO, CW], F32, name="q2_sb", tag="q2_sb")
            k2_sb = singles.tile([P, NO, CW], F32, name="k2_sb", tag="k2_sb")
            q2_src = q2_d.ap().rearrange("(o p) c -> p o c", p=P)
            k2_src = k2_d.ap().rearrange("(o p) c -> p o c", p=P)
            nc.gpsimd.dma_start(q2_sb[:, 0:8, :], q2_src[:, 0:8, :])
            nc.gpsimd.dma_start(q2_sb[:, 8:16, :], q2_src[:, 8:16, :])
            for g in range(4):
                nc.gpsimd.dma_start(k2_sb[:, g_sl(g), :], k2_src[:, g_sl(g), :])

        # SP HW-DGE: v groups 0 and 2 (serial on the SP channel: ~7.5us and
        # ~14us arrival — in time for acc t=0..3 / t=8..11).
        nc.sync.dma_start(v_sb[:, g_sl(0), :], v_src[:, g_sl(0), :])
        nc.sync.dma_start(v_sb[:, g_sl(2), :], v_src[:, g_sl(2), :])

        # f32 identity + remaining weight-tile zero-halves on GpSimd, after
        # the triggers (first consumer of kTzb0 is head 1 at ~30us; of pair 1
        # head 2 at ~50us; ident at ~35us).
        kTz_all = []
        for jp in range(NH4 // 2):
            kTza = tpool.tile([P, NT], MM_DT, tag="kTza", name="kTza")
            kTzb = tpool.tile([P, NT], MM_DT, tag="kTzb", name="kTzb")
            kTz_all.extend([kTza, kTzb])
        ident = singles.tile([P, P], F32)
        make_identity(nc, ident)
        nc.gpsimd.memset(kTz_all[1][0:HD, :].bitcast(U32), 0)
        nc.gpsimd.memset(kTz_all[2][HD:P, :].bitcast(U32), 0)
        nc.gpsimd.memset(kTz_all[3][0:HD, :].bitcast(U32), 0)

        # V' = [V | ones]: the ones column for all heads in one memset.
        v1r = singles.tile([P, NO, NH4, HD + 1], MM_DT)
        nc.gpsimd.memset(v1r[:, :, :, HD : HD + 1], 1.0)

        # DVE: zero-half of the first weight tile (gates st(0), so it leads
        # the DVE queue), then per-group casts + transpose staging copies.
        nc.vector.memset(kTz_all[0][HD:P, :].bitcast(U32), 0)

        qbf = singles.tile([P, NO, CW], BF16)
        kbf = singles.tile([P, NO, CW], BF16)
        qT2s = [
            tpool.tile([P, NT], MM_DT, tag="qT2", name="qT2") for _ in range(2)
        ]

        # ---- stage B: stacked transposes for BOTH head pairs per cast
        # group: qT2[jp] [128, 2048] holds heads 2jp (partitions 0:64) and
        # 2jp+1 (64:128); kT is split into two zero-padded weight tensors so
        # the logit matmuls contract over the full 128 partitions. Emitting
        # both pairs here lets all of it run in the DMA-bound startup window.
        for g in range(NO // 4):
            gsl = g_sl(g)
            nc.vector.tensor_copy(qbf[:, gsl, :], q_sb[:, gsl, :])
            nc.vector.tensor_copy(kbf[:, gsl, :], k_sb[:, gsl, :])
            for jp in range(2):
                cp = 2 * HD * jp
                tp = tr_tile([P, 4 * P], BF16)
                for u in range(4):
                    o = 4 * g + u
                    nc.tensor.transpose(
                        tp[:, P * u : P * (u + 1)], qbf[:, o, cp : cp + P], ident_b
                    )
                nc.vector.tensor_copy(qT2s[jp][:, 4 * P * g : 4 * P * (g + 1)], tp)
                tp2 = tr_tile([P, 4 * P], BF16)
                for u in range(4):
                    o = 4 * g + u
                    nc.tensor.transpose(
                        tp2[:, P * u : P * (u + 1)], kbf[:, o, cp : cp + P], ident_b
                    )
                nc.vector.tensor_copy(
                    kTz_all[2 * jp][0:HD, 4 * P * g : 4 * P * (g + 1)], tp2[0:HD]
                )
                nc.vector.tensor_copy(
                    kTz_all[2 * jp + 1][HD:P, 4 * P * g : 4 * P * (g + 1)], tp2[HD:P]
                )

        # V' value columns: head 0 per-group (so acc t can start as each v
        # group lands), heads 1-3 as whole copies (DVE backfills ~15-20us).
        for g in range(NO // 4):
            nc.vector.tensor_copy(
                v1r[:, g_sl(g), 0, 0:HD], v_sb[:, g_sl(g), 0:HD]
            )
        for j in range(1, NH4):
            ch = HD * j
            nc.vector.tensor_copy(v1r[:, :, j, 0:HD], v_sb[:, :, ch : ch + HD])

        # ---- gate factors for all heads at once (emitted at head-1
        # priority inside the j-loop, so head 0's startup work always
        # outranks it; data-gates on k2/q2 make it run ~20-30us in, well
        # before the first phase2 reads gte_all) ----
        gte_all = None

        def emit_gate_all():
            k2o = singles.tile([P, CW], F32)
            k2b_sb = singles.tile([P, CW], F32)
            for cc in range(4):
                csl = slice(HD * cc, HD * (cc + 1))
                nc.vector.reduce_sum(
                    out=k2o[:, csl],
                    in_=k2_sb[:, :, csl].rearrange("p o c -> p c o"),
                    axis=mybir.AxisListType.X,
                )
            nc.gpsimd.partition_all_reduce(
                k2b_sb, k2o, channels=P, reduce_op=bass_isa.ReduceOp.add
            )
            z_all = singles.tile([P, NO, NH4], F32)
            for j in range(NH4):
                ch = HD * j
                zt = opool.tile([P, NO, HD], F32, tag="zt", name="zt")
                nc.vector.tensor_mul(
                    zt,
                    q2_sb[:, :, ch : ch + HD],
                    k2b_sb[:, None, ch : ch + HD].to_broadcast((P, NO, HD)),
                )
                nc.vector.reduce_sum(
                    out=z_all[:, :, j], in_=zt, axis=mybir.AxisListType.X
                )
            eg_all = singles.tile([P, NO, NH4], F32)
            nc.scalar.activation(
                eg_all, z_all, mybir.ActivationFunctionType.Exp, scale=-SCALE
            )
            nc.vector.tensor_scalar_add(eg_all, eg_all, 1.0)
            g_t = singles.tile([P, NO, NH4], F32)
            nc.vector.reciprocal(g_t, eg_all)
            return g_t

        out_ap3 = out_d.ap().rearrange("(o p) c -> p o c", p=P)

        # ---- main loop: per (head, half): 16x [st matmuls -> exp -> acc
        # matmuls], then O^T copy-out + phase2 (transpose + gate/rowsum
        # scale + store). phase2/out drain in the shadow of the next half's
        # exp stream; the last half drains at fine grain to shrink the tail.
        for j in range(NH4):  # local head
            jp, jj = divmod(j, 2)
            ch = HD * j
            qT2 = qT2s[jp]
            kTz = kTz_all[2 * jp + jj]
            for h in range(NHALF):  # q half
                last = j == NH4 - 1 and h == NHALF - 1
                q0 = QH * h
                acc = ps_ac.tile([HD + 1, QH], F32, tag="pac")
                for t in range(NO):
                    st = ps_st.tile([P, QH], F32, tag="pst")
                    for s2 in range(QH // 512):
                        nc.tensor.matmul(
                            st[:, 512 * s2 : 512 * (s2 + 1)],
                            kTz[:, P * t : P * (t + 1)],
                            qT2[:, q0 + 512 * s2 : q0 + 512 * (s2 + 1)],
                            start=True,
                            stop=True,
                        )
                    et = epool.tile([P, QH], MM_DT, tag="et")
                    nc.scalar.activation(
                        et, st, mybir.ActivationFunctionType.Exp, scale=SCALE
                    )
                    for s2 in range(QH // 512):
                        nc.tensor.matmul(
                            acc[:, 512 * s2 : 512 * (s2 + 1)],
                            v1r[:, t, j, :],
                            et[:, 512 * s2 : 512 * (s2 + 1)],
                            start=(t == 0),
                            stop=(t == NO - 1),
                        )

                if use_sigmoid and j == 0 and h == 0:
                    # gate chain lands here in priority: below all startup
                    # and (0,h0) work, above everything later; its k2/q2
                    # data-gates let it fill DVE idle during (0,h0)'s
                    # exp-bound stream.
                    gte_all = emit_gate_all()

                # O^T -> SBUF (+ fused rowsum/gate scale after transpose)
                ot_sb = opool.tile([HD + 1, QH], F32, tag="ot", bufs=3, name="ot_sb")
                obuf = opool.tile([P, QH // P, HD], F32, tag="obuf")
                if last:
                    # per-u copies so the drain pipeline starts 1 tile in
                    for u in range(QH // P):
                        nc.vector.tensor_copy(
                            ot_sb[:, P * u : P * (u + 1)], acc[:, P * u : P * (u + 1)]
                        )
                else:
                    nc.vector.tensor_copy(ot_sb, acc)

                chunk = 2 if last else 4  # o-tiles per output DMA
                for u in range(QH // P):
                    i = (QH // P) * h + u
                    tr = tr_tile([P, HD + 1])
                    nc.tensor.transpose(
                        tr, ot_sb[:, P * u : P * (u + 1)], ident[: HD + 1, : HD + 1]
                    )
                    rcp = opool.tile([P, 1], F32, tag="rcp", name="rcp")
                    nc.vector.reciprocal(rcp, tr[:, HD : HD + 1])
                    if use_sigmoid:
                        fac = opool.tile([P, 1], F32, tag="fac", name="fac")
                        nc.vector.tensor_mul(fac, rcp, gte_all[:, i, j : j + 1])
                    else:
                        fac = rcp
                    nc.vector.tensor_scalar_mul(obuf[:, u, :], tr[:, 0:HD], fac)
                    if u % chunk == chunk - 1:
                        c0 = u - chunk + 1
                        nc.sync.dma_start(
                            out_ap3[:, 8 * h + c0 : 8 * h + u + 1, ch : ch + HD],
                            obuf[:, c0 : u + 1, :],
                        )

    nc.compile()
    return nc


@functools.lru_cache(maxsize=2)
def _graph(use_sigmoid: bool):
    return _build(use_sigmoid)


def _shard(a: np.ndarray, i: int) -> np.ndarray:
    b, hg = divmod(i, 2)
    return np.ascontiguousarray(a[b, :, hg * CW : (hg + 1) * CW], dtype=np.float32)


def run(inputs, trace: bool = False):
    use_sigmoid = bool(np.asarray(inputs["use_sigmoid"]).item())
    nc = _graph(use_sigmoid)
    in_maps = []
    for i in range(8):
        m = {
            "q": _shard(np.asarray(inputs["query"]), i),
            "k": _shard(np.asarray(inputs["key"]), i),
            "v": _shard(np.asarray(inputs["value"]), i),
        }
        if use_sigmoid:
            m["q2"] = _shard(np.asarray(inputs["query2"]), i)
            m["k2"] = _shard(np.asarray(inputs["key2"]), i)
        in_maps.append(m)
    res = bass_utils.run_bass_kernel_spmd(
        nc, in_maps, core_ids=list(range(8)), trace=trace
    )
    out = np.empty((B, NT, C), dtype=np.float32)
    for i in range(8):
        b, hg = divmod(i, 2)
        out[b, :, hg * CW : (hg + 1) * CW] = res.results[i]["out"]
    return out, res


def kernel(**inputs) -> np.ndarray:
    out, _ = run(inputs)
    return out


if __name__ == "__main__":
    rng = np.random.default_rng(0)
    fake = {
        "query": rng.standard_normal((B, NT, C), dtype=np.float32),
        "key": rng.standard_normal((B, NT, C), dtype=np.float32),
        "value": rng.standard_normal((B, NT, C), dtype=np.float32),
        "query2": rng.standard_normal((B, NT, C), dtype=np.float32),
        "key2": rng.standard_normal((B, NT, C), dtype=np.float32),
        "use_sigmoid": 1,
    }
    out = kernel(**fake)
    print("ran ok", out.shape, out.dtype)
